# revision 6
# baseline (speedup 1.0000x reference)
"""Bass/Trainium2 kernel for nn_Differential_Attention_60825326846200.

Mathematical reduction of the reference:
  scores[b,h,i,j] = (sum_d q[b,h,i,d] - k[b,h,i,d]) / sqrt(DH) + mask[b,i]
is constant over the key index j, so the softmax over j is exactly the
uniform distribution (1/S) regardless of q, k, and the mask.  Hence
  ctx[b,h,i,:] = mean_j v[b,h,j,:]          (independent of i)
  out[b,i,:]   = (mean_j hidden_b[b,j,:]) @ Wv.T + bv   for every i.
The q/k projections and the attention mask cancel exactly.

Distribution across the 8 NeuronCores — two collective-free SPMD launches
(a cross-core AllReduce measures 130us+ in barrier+mesh latency here; two
launch fixed-overheads are far cheaper).  Host glue between launches is
pure data movement (slice/permute/stack).

  Launch 1 (mean, sequence-sharded): core c gets its [B, S/8, HID] slice
  of hidden_b host-permuted to [128(p), KC(kc), B, S/8] (k = kc*128+p on
  the partition axis), streamed in 8 chunks alternating the two HWDGE
  rings, each reduced over the sequence axis by a pipelined DVE
  reduce_sum -> 8KB partial-sum output "part" [128, KC*B].

  Launch 2 (projection, feature-sharded): core c produces the output
  feature slice o in [128c, 128c+128).  The projection contraction
  out_row[b,o] = sum_kc sum_p hbT[p,kc*B+b] * wvt[p,kc,o] runs as:
    - DVE: per kc, prod[p,(b,o,kc)] = wvt[p,kc,o] * hbT[p,kc*B+b] in
      bf16 (broadcast-read APs, no materialized operands), chasing the
      two wvt DMA rings chunk by chunk;
    - DVE/Pool: per batch, reduce prod over kc -> red_b [128, O] bf16
      (vector takes b=0, gpsimd b=1, in parallel);
    - PE: one ones-stationary matmul per batch reduces red_b across
      partitions AND broadcasts the result to all 128 output partitions
      in the same instruction (psum_b[p,o] = sum_k red_b[k,o]).  The
      S*bv bias rides a separate early accumulating matmul (start=True)
      into the same PSUM region, off the critical path.
  The PSUM->SBUF copies scale by 1/S (exact, S = 2^11) into a single
  [128, O] tile whose partition p holds the row for batch p//64; the
  [B,S,O] output slice is then written as ONE contiguous 16KB run per
  partition (dest p covers 32 consecutive sequence rows of batch p//64),
  split across the two HWDGE rings.

Host does data movement only: slicing/permutation/stacking.
"""

import numpy as np

import concourse.bacc as bacc
import concourse.mybir as mybir
import concourse.tile as tile
from concourse.bass_utils import run_bass_kernel_spmd

N_CORES = 8
B, S, HID = 2, 2048, 1024
S_LOC = S // N_CORES  # 256 sequence positions reduced per core
O_LOC = HID // N_CORES  # 128 output features produced per core
KC = HID // 128  # 8 contraction chunks of 128
F32 = mybir.dt.float32
BF16 = mybir.dt.bfloat16

_compiled = None


def _new_nc():
    return bacc.Bacc(
        "TRN2",
        target_bir_lowering=False,
        debug=False,
        enable_asserts=False,
        num_devices=N_CORES,
    )


def _build_mean():
    """Launch 1: partial column-sum of this core's hidden_b slice.
    Input "hbt" [128, KC, B, S_LOC]: hbt[p, kc, b, s] = hb[b, s, kc*128+p].
    Output "part" [128, KC*B] with column kc*B + b (raw sums, unscaled)."""
    nc = _new_nc()
    hbt = nc.dram_tensor("hbt", [128, KC, B, S_LOC], F32, kind="ExternalInput").ap()
    part = nc.dram_tensor("part", [128, KC * B], F32, kind="ExternalOutput").ap()

    nch = 8  # DMA/reduce pipeline chunks (one kc each), 2 HWDGE rings
    kpc = KC // nch  # kc per chunk
    chunk = kpc * B * S_LOC  # free elements per chunk
    with tile.TileContext(nc) as tc:
        with (
            tc.tile_pool(name="big", bufs=1) as big,
            tc.tile_pool(name="small", bufs=1) as small,
        ):
            part_sb = small.tile([128, KC * B], F32)
            for h in range(nch):
                eng = nc.sync if h % 2 == 0 else nc.scalar
                t_sb = big.tile([128, chunk], F32, name=f"hbsb{h}")
                eng.dma_start(
                    t_sb[:].rearrange("p (kc b s) -> p kc b s", kc=kpc, b=B),
                    hbt[:, h * kpc : (h + 1) * kpc],
                )
                nc.vector.reduce_sum(
                    part_sb[:, h * kpc * B : (h + 1) * kpc * B],
                    t_sb[:].rearrange("p (kc b s) -> p kc b s", kc=kpc, b=B),
                    axis=mybir.AxisListType.X,
                )
                if h == nch - 2:
                    # overlap most of the tiny result write with the tail;
                    # sync ring: its small-transfer receipts are 1-2us
                    # faster than scalar's, and the tail drain waits on them
                    nc.sync.dma_start(
                        part[:, : (nch - 1) * kpc * B],
                        part_sb[:, : (nch - 1) * kpc * B],
                    )
            nc.sync.dma_start(
                part[:, (nch - 1) * kpc * B :], part_sb[:, (nch - 1) * kpc * B :]
            )
    nc.compile()
    return nc


def _build_proj():
    """Launch 2: sum the 8 partials, project through this core's
    (host-pre-transposed) Wv rows in bf16, add bias, reduce+broadcast
    across partitions on the tensor engine, write the [B, S, O_LOC]
    output slice as one 16KB contiguous run per partition.
    Input "wvt" [128, KC, O_LOC]: wvt[p, kc, o] = Wv[c*128+o, kc*128+p]."""
    nc = _new_nc()
    parts = nc.dram_tensor(
        "parts", [128, KC * B, N_CORES], F32, kind="ExternalInput"
    ).ap()
    wvt = nc.dram_tensor("wvt", [128, KC, O_LOC], F32, kind="ExternalInput").ap()
    bv = nc.dram_tensor("bv", [1, O_LOC], F32, kind="ExternalInput").ap()
    out = nc.dram_tensor("out", [B, S, O_LOC], F32, kind="ExternalOutput").ap()

    TPB = S // 64  # sequence rows per partition in the blocked write (32)

    with tile.TileContext(nc) as tc:
        with (
            tc.tile_pool(name="big", bufs=1) as big,
            tc.tile_pool(name="small", bufs=1) as small,
            tc.tile_pool(name="psum", bufs=1, space="PSUM") as psum,
        ):
            # ---- input DMAs.  sync ring: parts first (gates the combine),
            # then wvt kc 0-3 in two 128KB chunks; scalar ring: wvt kc 4-7
            # in two chunks, then bv.  Output writes queue last on each ring.
            parts_sb = small.tile([128, KC * B * N_CORES], F32)
            nc.sync.dma_start(parts_sb[:], parts[:])
            wvT = big.tile([128, KC * O_LOC], F32)
            qk = KC // 4  # kc per DMA chunk (2)
            for q, eng in enumerate((nc.sync, nc.sync, nc.scalar, nc.scalar)):
                eng.dma_start(
                    wvT[:, q * qk * O_LOC : (q + 1) * qk * O_LOC].rearrange(
                        "p (kc o) -> p kc o", kc=qk
                    ),
                    wvt[:, q * qk : (q + 1) * qk],
                )
            bv_sb = small.tile([1, O_LOC], F32)
            nc.scalar.dma_start(bv_sb[:], bv[:])

            # ---- constants (gpsimd, early, off the critical path)
            # blk[b][k, m] = 1 iff output partition m is in batch b's half;
            # used as matmul stationaries so each batch's partition
            # reduce+broadcast lands directly in its half of ONE psum tile
            blk = small.tile([128, B * 128], BF16, name="blk")
            nc.gpsimd.memset(blk[:, 0:64], 1.0)
            nc.gpsimd.memset(blk[:, 64:128], 0.0)
            nc.gpsimd.memset(blk[:, 128:192], 0.0)
            nc.gpsimd.memset(blk[:, 192:256], 1.0)
            # bias rhs: zeros except partition 0 = S*bv per batch column block
            rb = small.tile([128, B * O_LOC], BF16, name="rb")
            nc.gpsimd.memset(rb[:], 0.0)
            for b in range(B):
                nc.gpsimd.tensor_scalar_mul(
                    rb[0:1, b * O_LOC : (b + 1) * O_LOC], bv_sb[:], float(S)
                )

            # ---- combine the 8 partial sums (raw, unscaled)
            hbT = small.tile([128, KC * B], F32)
            nc.vector.reduce_sum(
                hbT[:],
                parts_sb[:].rearrange("p (c n) -> p c n", n=N_CORES),
                axis=mybir.AxisListType.X,
            )

            # ---- per-kc products in bf16, chasing the wvt DMA chunks
            prod = big.tile([128, B * O_LOC * KC], BF16, name="prod")
            prod_v = prod[:].rearrange("p (b o kc) -> p b o kc", b=B, kc=KC)
            for kc in range(KC):
                nc.vector.tensor_mul(
                    prod_v[:, :, :, kc],
                    wvT[:, kc * O_LOC : (kc + 1) * O_LOC]
                    .unsqueeze(1)
                    .broadcast_to([128, B, O_LOC]),
                    hbT[:, kc * B : (kc + 1) * B]
                    .unsqueeze(2)
                    .broadcast_to([128, B, O_LOC]),
                )

            # ---- bias matmuls first (start=True on the first, PE idle),
            # then the reduction matmuls accumulate on top; all four target
            # the SAME psum tile, each batch masked to its partition half
            # by the blk stationary
            pb = psum.tile([128, O_LOC], F32, name="pb", tag="pb")
            for b in range(B):
                nc.tensor.matmul(
                    pb[:],
                    lhsT=blk[:, b * 128 : (b + 1) * 128],
                    rhs=rb[:, b * O_LOC : (b + 1) * O_LOC],
                    start=(b == 0),
                    stop=False,
                )

            # ---- kc-reduction, both batches in one DVE op (gpsimd cannot
            # reduce over free axes)
            red = small.tile([128, B * O_LOC], BF16, name="red")
            with nc.allow_low_precision(reason="bf16 matmul feed, tol 2e-2"):
                nc.vector.reduce_sum(
                    red[:].rearrange("p (b o) -> p b o", b=B),
                    prod_v[:],
                    axis=mybir.AxisListType.X,
                )

            # ---- partition reduce+broadcast, masked per batch half:
            # pb[p, o] = sum_k red_{p//64}[k, o]  (+ S*bv from the bias MMs)
            for b in range(B):
                nc.tensor.matmul(
                    pb[:],
                    lhsT=blk[:, b * 128 : (b + 1) * 128],
                    rhs=red[:, b * O_LOC : (b + 1) * O_LOC],
                    start=False,
                    stop=(b == B - 1),
                )

            # ---- scale 1/S into the blocked write tile: partition p holds
            # the row for batch p//64
            tbc = big.tile([128, O_LOC], F32, name="tbc")
            nc.vector.tensor_scalar_mul(tbc[:], pb[:], 1.0 / S)

            # ---- blocked output write: dest partition p covers sequence
            # rows [TPB*(p%64), TPB*(p%64)+TPB) of batch p//64 -> one 16KB
            # contiguous run per partition, one DMA per HWDGE ring.
            dst = out.rearrange("b (p2 t) o -> (b p2) t o", t=TPB)
            src = tbc[:].unsqueeze(1).broadcast_to([128, TPB, O_LOC])
            nc.sync.dma_start(dst[0:64], src[0:64])
            nc.scalar.dma_start(dst[64:128], src[64:128])
    nc.compile()
    return nc


def get_ncs():
    global _compiled
    if _compiled is None:
        _compiled = (_build_mean(), _build_proj())
    return _compiled


def make_mean_in_maps(inputs):
    hb = np.asarray(inputs["hidden_states_b"], dtype=np.float32)
    # [B, S, HID] -> per core [128, KC, B, S_LOC] (pure permutation)
    maps = []
    for c in range(N_CORES):
        sl = hb[:, c * S_LOC : (c + 1) * S_LOC, :]  # [B, S_LOC, HID]
        t = sl.reshape(B, S_LOC, KC, 128).transpose(3, 2, 0, 1)
        maps.append({"hbt": np.ascontiguousarray(t)})
    return maps


def make_proj_in_maps(inputs, part_results):
    Wv = np.asarray(inputs["Wv"], dtype=np.float32)
    bv = np.asarray(inputs["bv"], dtype=np.float32)
    parts = np.ascontiguousarray(
        np.stack([part_results[c]["part"] for c in range(N_CORES)], axis=-1)
    )
    maps = []
    for c in range(N_CORES):
        w = Wv[c * O_LOC : (c + 1) * O_LOC, :]  # [O_LOC, HID]
        wt = w.reshape(O_LOC, KC, 128).transpose(2, 1, 0)  # [128, KC, O_LOC]
        maps.append(
            {
                "parts": parts,
                "wvt": np.ascontiguousarray(wt),
                "bv": np.ascontiguousarray(
                    bv[c * O_LOC : (c + 1) * O_LOC].reshape(1, O_LOC)
                ),
            }
        )
    return maps


def gather_out(results):
    return np.concatenate([results[c]["out"] for c in range(N_CORES)], axis=2)


def kernel(**inputs) -> np.ndarray:
    nc_mean, nc_proj = get_ncs()
    cores = list(range(N_CORES))
    res1 = run_bass_kernel_spmd(nc_mean, make_mean_in_maps(inputs), cores)
    res2 = run_bass_kernel_spmd(nc_proj, make_proj_in_maps(inputs, res1.results), cores)
    return gather_out(res2.results)


# revision 11
# speedup vs baseline: 1.1210x; 1.1210x over previous
"""Bass/Trainium2 kernel for nn_Differential_Attention_60825326846200.

Mathematical reduction of the reference:
  scores[b,h,i,j] = (sum_d q[b,h,i,d] - k[b,h,i,d]) / sqrt(DH) + mask[b,i]
is constant over the key index j, so the softmax over j is exactly the
uniform distribution (1/S) regardless of q, k, and the mask.  Hence
  ctx[b,h,i,:] = mean_j v[b,h,j,:]          (independent of i)
  out[b,i,:]   = (mean_j hidden_b[b,j,:]) @ Wv.T + bv   for every i.
The q/k projections and the attention mask cancel exactly.

Distribution across the 8 NeuronCores — two collective-free SPMD launches
(a cross-core AllReduce measures 130us+ in barrier+mesh latency here; two
launch fixed-overheads are far cheaper).  Host glue between launches is
pure data movement (slice/permute/stack).

  Launch 1 (mean, sequence-sharded): core c gets its [B, S/8, HID] slice
  of hidden_b host-permuted to [128(p), KC(kc), B, S/8] (k = kc*128+p on
  the partition axis), streamed in 8 chunks alternating the two HWDGE
  rings, each reduced over the sequence axis by a pipelined DVE
  reduce_sum -> 8KB partial-sum output "part" [128, KC*B].

  Launch 2 (projection, feature-sharded): core c produces the output
  feature slice o in [128c, 128c+128).  The projection contraction
  out_row[b,o] = sum_kc sum_p hbT[p,kc*B+b] * wvt[p,kc,o] runs as:
    - DVE: per kc, prod[p,(b,o,kc)] = wvt[p,kc,o] * hbT[p,kc*B+b] in
      bf16 (broadcast-read APs, no materialized operands), chasing the
      two wvt DMA rings chunk by chunk;
    - DVE/Pool: per batch, reduce prod over kc -> red_b [128, O] bf16
      (vector takes b=0, gpsimd b=1, in parallel);
    - PE: one ones-stationary matmul per batch reduces red_b across
      partitions AND broadcasts the result to all 128 output partitions
      in the same instruction (psum_b[p,o] = sum_k red_b[k,o]).  The
      S*bv bias rides a separate early accumulating matmul (start=True)
      into the same PSUM region, off the critical path.
  The PSUM->SBUF copies scale by 1/S (exact, S = 2^11) into a single
  [128, O] tile whose partition p holds the row for batch p//64; the
  [B,S,O] output slice is then written as ONE contiguous 16KB run per
  partition (dest p covers 32 consecutive sequence rows of batch p//64),
  split across the two HWDGE rings.

Host does data movement only: slicing/permutation/stacking.
"""

import numpy as np

import concourse.bacc as bacc
import concourse.mybir as mybir
import concourse.tile as tile
from concourse.bass_utils import run_bass_kernel_spmd

N_CORES = 8
B, S, HID = 2, 2048, 1024
S_LOC = S // N_CORES  # 256 sequence positions reduced per core
O_LOC = HID // N_CORES  # 128 output features produced per core
KC = HID // 128  # 8 contraction chunks of 128
F32 = mybir.dt.float32
BF16 = mybir.dt.bfloat16

_compiled = None


def _new_nc():
    return bacc.Bacc(
        "TRN2",
        target_bir_lowering=False,
        debug=False,
        enable_asserts=False,
        num_devices=N_CORES,
    )


def _build_mean():
    """Launch 1: partial column-sum of this core's hidden_b slice.
    Input "hbt" [128, KC, B, S_LOC]: hbt[p, kc, b, s] = hb[b, s, kc*128+p].
    Output "part" [128, KC*B] with column kc*B + b (raw sums, unscaled)."""
    nc = _new_nc()
    hbt = nc.dram_tensor("hbt", [128, KC, B, S_LOC], F32, kind="ExternalInput").ap()
    part = nc.dram_tensor("part", [128, KC * B], F32, kind="ExternalOutput").ap()

    nch = 8  # DMA/reduce pipeline chunks (one kc each), 2 HWDGE rings
    kpc = KC // nch  # kc per chunk
    chunk = kpc * B * S_LOC  # free elements per chunk
    with tile.TileContext(nc) as tc:
        with (
            tc.tile_pool(name="big", bufs=1) as big,
            tc.tile_pool(name="small", bufs=1) as small,
        ):
            part_sb = small.tile([128, KC * B], F32)
            for h in range(nch):
                eng = nc.sync if h % 2 == 0 else nc.scalar
                t_sb = big.tile([128, chunk], F32, name=f"hbsb{h}")
                eng.dma_start(
                    t_sb[:].rearrange("p (kc b s) -> p kc b s", kc=kpc, b=B),
                    hbt[:, h * kpc : (h + 1) * kpc],
                )
                nc.vector.reduce_sum(
                    part_sb[:, h * kpc * B : (h + 1) * kpc * B],
                    t_sb[:].rearrange("p (kc b s) -> p kc b s", kc=kpc, b=B),
                    axis=mybir.AxisListType.X,
                )
                if h == nch - 2:
                    # overlap most of the tiny result write with the tail;
                    # sync ring: its small-transfer receipts are 1-2us
                    # faster than scalar's, and the tail drain waits on them
                    nc.sync.dma_start(
                        part[:, : (nch - 1) * kpc * B],
                        part_sb[:, : (nch - 1) * kpc * B],
                    )
            nc.sync.dma_start(
                part[:, (nch - 1) * kpc * B :], part_sb[:, (nch - 1) * kpc * B :]
            )
    nc.compile()
    return nc


def _build_proj():
    """Launch 2: sum the 8 partials, project through this core's
    (host-pre-transposed) Wv rows in bf16, add bias, reduce+broadcast
    across partitions on the tensor engine, write the [B, S, O_LOC]
    output slice as one 16KB contiguous run per partition.
    Input "wvt" [128, KC, O_LOC]: wvt[p, kc, o] = Wv[c*128+o, kc*128+p]."""
    nc = _new_nc()
    parts = nc.dram_tensor(
        "parts", [128, KC * B, N_CORES], F32, kind="ExternalInput"
    ).ap()
    wvt = nc.dram_tensor("wvt", [128, KC, O_LOC], F32, kind="ExternalInput").ap()
    bv = nc.dram_tensor("bv", [1, O_LOC], F32, kind="ExternalInput").ap()
    out = nc.dram_tensor("out", [B, S, O_LOC], F32, kind="ExternalOutput").ap()

    TPB = S // 64  # sequence rows per partition in the blocked write (32)

    with tile.TileContext(nc) as tc:
        with (
            tc.tile_pool(name="big", bufs=1) as big,
            tc.tile_pool(name="small", bufs=1) as small,
            tc.tile_pool(name="psum", bufs=1, space="PSUM") as psum,
        ):
            # ---- input DMAs.  scalar ring: parts first (gates the combine),
            # then wvt kc 4-7 in two 128KB chunks, then bv; sync ring: wvt
            # kc 0-3 in two chunks (first chunk gates the MAC chains, so it
            # issues immediately).  Output writes queue last on each ring.
            parts_sb = small.tile([128, KC * B * N_CORES], F32)
            nc.scalar.dma_start(parts_sb[:], parts[:])
            wvT = big.tile([128, KC * O_LOC], F32)
            qk = KC // 4  # kc per DMA chunk (2)
            for q, eng in enumerate((nc.sync, nc.sync, nc.scalar, nc.scalar)):
                eng.dma_start(
                    wvT[:, q * qk * O_LOC : (q + 1) * qk * O_LOC].rearrange(
                        "p (kc o) -> p kc o", kc=qk
                    ),
                    wvt[:, q * qk : (q + 1) * qk],
                )
            bv_sb = small.tile([1, O_LOC], F32)
            nc.scalar.dma_start(bv_sb[:], bv[:])

            # ---- constants (gpsimd, early, off the critical path)
            # blk[b][k, m] = 1 iff output partition m is in batch b's half;
            # used as matmul stationaries so each batch's partition
            # reduce+broadcast lands directly in its half of ONE psum tile
            blk = small.tile([128, B * 128], F32, name="blk")
            nc.gpsimd.memset(blk[:, 0:64], 1.0)
            nc.gpsimd.memset(blk[:, 64:128], 0.0)
            nc.gpsimd.memset(blk[:, 128:192], 0.0)
            nc.gpsimd.memset(blk[:, 192:256], 1.0)
            # bias rhs: zeros except partition 0 = bv per batch column block
            # (single-partition ops are poison on gpsimd: build rows on DVE)
            rb = small.tile([128, B * O_LOC], F32, name="rb")
            nc.gpsimd.memset(rb[:], 0.0)
            for b in range(B):
                nc.vector.tensor_scalar_mul(
                    rb[0:1, b * O_LOC : (b + 1) * O_LOC], bv_sb[:], 1.0
                )

            # ---- combine the 8 partial sums, then scale by 1/S (exact,
            # S = 2^11) so the MAC chains and PSUM are in output scale
            hbT = small.tile([128, KC * B], F32)
            nc.vector.reduce_sum(
                hbT[:],
                parts_sb[:].rearrange("p (c n) -> p c n", n=N_CORES),
                axis=mybir.AxisListType.X,
            )
            nc.vector.tensor_scalar_mul(hbT[:], hbT[:], 1.0 / S)

            # ---- bias matmuls first (start=True on the first, PE idle),
            # then the reduction matmuls accumulate on top; all four target
            # the SAME psum tile, each batch masked to its partition half
            # by the blk stationary
            pb = psum.tile([128, O_LOC], F32, name="pb", tag="pb")
            for b in range(B):
                nc.tensor.matmul(
                    pb[:],
                    lhsT=blk[:, b * 128 : (b + 1) * 128],
                    rhs=rb[:, b * O_LOC : (b + 1) * O_LOC],
                    start=(b == 0),
                    stop=False,
                )

            # ---- projection MAC chains with contiguous APs and the
            # per-partition (pre-scaled) hbar column as the scalar operand:
            #   acc_b = sum_kc wvT[:, kc, :] * hbTs[:, kc*B+b]
            # Batch 0: fused multiply-accumulate chain on vector (the only
            # engine with per-partition-scalar MACs).  Batch 1: products on
            # the Act engine (Copy activation with per-partition scale),
            # accumulated by Pool tensor_adds — three engines in parallel,
            # all chasing the wvt DMA chunks.
            acc = big.tile([128, B * O_LOC], F32, name="acc")
            prod1 = big.tile([128, KC * O_LOC], F32, name="prod1")
            for kc in range(KC):
                w = wvT[:, kc * O_LOC : (kc + 1) * O_LOC]
                # batch 0 on vector
                a0 = acc[:, 0:O_LOC]
                h0 = hbT[:, kc * B : kc * B + 1]
                if kc == 0:
                    nc.vector.tensor_scalar_mul(a0, w, h0)
                else:
                    nc.vector.scalar_tensor_tensor(
                        a0, w, h0, a0,
                        op0=mybir.AluOpType.mult, op1=mybir.AluOpType.add,
                    )
                # batch 1 on Act (product) + Pool (accumulate)
                a1 = acc[:, O_LOC:]
                p1 = prod1[:, kc * O_LOC : (kc + 1) * O_LOC]
                h1 = hbT[:, kc * B + 1 : kc * B + 2]
                nc.scalar.activation(
                    p1, w, mybir.ActivationFunctionType.Copy, scale=h1
                )
                if kc == 0:
                    nc.gpsimd.tensor_copy(a1, p1)
                else:
                    nc.gpsimd.tensor_add(a1, a1, p1)

            # ---- partition reduce+broadcast, masked per batch half:
            # pb[p, o] = sum_k acc_{p//64}[k, o]  (+ bv from the bias MMs)
            for b in range(B):
                nc.tensor.matmul(
                    pb[:],
                    lhsT=blk[:, b * 128 : (b + 1) * 128],
                    rhs=acc[:, b * O_LOC : (b + 1) * O_LOC],
                    start=False,
                    stop=(b == B - 1),
                )

            # ---- blocked write tile: partition p holds the row for batch
            # p//64 (pb is already in output scale)
            tbc = big.tile([128, O_LOC], F32, name="tbc")
            nc.vector.tensor_scalar_mul(tbc[:], pb[:], 1.0)

            # ---- blocked output write: dest partition p covers sequence
            # rows [TPB*(p%64), TPB*(p%64)+TPB) of batch p//64 -> one 16KB
            # contiguous run per partition, one DMA per HWDGE ring.
            dst = out.rearrange("b (p2 t) o -> (b p2) t o", t=TPB)
            src = tbc[:].unsqueeze(1).broadcast_to([128, TPB, O_LOC])
            nc.sync.dma_start(dst[0:64], src[0:64])
            nc.scalar.dma_start(dst[64:128], src[64:128])
    nc.compile()
    return nc


def get_ncs():
    global _compiled
    if _compiled is None:
        _compiled = (_build_mean(), _build_proj())
    return _compiled


def make_mean_in_maps(inputs):
    hb = np.asarray(inputs["hidden_states_b"], dtype=np.float32)
    # [B, S, HID] -> per core [128, KC, B, S_LOC] (pure permutation)
    maps = []
    for c in range(N_CORES):
        sl = hb[:, c * S_LOC : (c + 1) * S_LOC, :]  # [B, S_LOC, HID]
        t = sl.reshape(B, S_LOC, KC, 128).transpose(3, 2, 0, 1)
        maps.append({"hbt": np.ascontiguousarray(t)})
    return maps


def make_proj_in_maps(inputs, part_results):
    Wv = np.asarray(inputs["Wv"], dtype=np.float32)
    bv = np.asarray(inputs["bv"], dtype=np.float32)
    parts = np.ascontiguousarray(
        np.stack([part_results[c]["part"] for c in range(N_CORES)], axis=-1)
    )
    maps = []
    for c in range(N_CORES):
        w = Wv[c * O_LOC : (c + 1) * O_LOC, :]  # [O_LOC, HID]
        wt = w.reshape(O_LOC, KC, 128).transpose(2, 1, 0)  # [128, KC, O_LOC]
        maps.append(
            {
                "parts": parts,
                "wvt": np.ascontiguousarray(wt),
                "bv": np.ascontiguousarray(
                    bv[c * O_LOC : (c + 1) * O_LOC].reshape(1, O_LOC)
                ),
            }
        )
    return maps


def gather_out(results):
    return np.concatenate([results[c]["out"] for c in range(N_CORES)], axis=2)


def kernel(**inputs) -> np.ndarray:
    nc_mean, nc_proj = get_ncs()
    cores = list(range(N_CORES))
    res1 = run_bass_kernel_spmd(nc_mean, make_mean_in_maps(inputs), cores)
    res2 = run_bass_kernel_spmd(nc_proj, make_proj_in_maps(inputs, res1.results), cores)
    return gather_out(res2.results)
